# revision 45
# baseline (speedup 1.0000x reference)
"""Trainium2 Bass kernel: transformer block (biased attention + residual).

Reference math (B=4, S=1024, H=1024, NH=16, DK=64):
    q = x_q @ Wq.T ; k = x_kv @ Wk.T ; v = x_kv @ Wv.T   (per-head reshape)
    scores = q k^T / sqrt(DK) + bias ; attn = softmax(scores)
    out = x_q + (attn v reshaped) @ Wo.T

Sharding: 8 cores = 4 batches x 2 head-groups (8 heads each). Each core
computes its (batch, head-group) slice; the host sums the two head-group
partial outputs per batch and adds the residual.

Per-core dataflow (all matmul inputs bf16, PSUM accumulation fp32):
    qT/kT = W_g x^T            (head_dim on partitions, seq on free)
    v     = x_kv @ Wv_g.T      (seq on partitions), padded ones column
    sT[k,q] = k_h q_h^T        (per head: both qc halves, then exp —
                               the ps_s pool turns over once per step, so
                               head i's matmuls only WAR against head i's
                               exp of the previous step: a full exp-slot
                               of slack)
    eT  = exp(sT / 8)          (ACT, scale folded into the activation)
    aT  = eT * exp(bias)^T     (DVE bf16 2x; exp(bias) precomputed host)
    avT = v_aug^T aT           -> rows 0..63 = attn out^T, row 64 = denom
                               (heads 0-5: discrete 8-matmul units one
                               pair behind; heads 6-7: accumulated
                               incrementally during pair 3 so the tail
                               has no bulk attn@v left)
    aoT = avT[0:64] * bcast(1/denom)   (1/denom broadcast across the 64
                                        head-dim partitions by a 0-stride
                                        DMA; no PE involvement)
    yT  = Wo_g^T-contraction of aoT    (tail units; y copies alternate
                                        DVE/ACT)
"""

import sys

import numpy as np

for _p in ("/opt/trn_rl_repo",):
    if _p not in sys.path:
        sys.path.append(_p)

B, S, H, NH = 4, 1024, 1024, 16
DK = 64
P = 128
NH_L = 8            # heads per core
JL = NH_L * DK      # 512 local head dims per core
FT = H // P         # 8 contraction tiles for projections
TT = S // P         # 8 seq tiles
JC = JL // P        # 4 local head-dim chunks of 128
QF = 512            # matmul moving free dim (one PSUM bank of fp32)
QC = S // QF        # 2 q chunks
N_CORES = 8


def _split_waits(nc, max_waits=1):
    """This walrus build rejects instructions carrying more than ~1 sem
    wait ("Too many sync wait commands" in setupSyncWait). Hoist surplus
    waits onto same-engine NoOps spliced immediately before the carrying
    instruction — same engine position, so semantics are unchanged."""
    import bass_rust
    import concourse.mybir as mybir

    n = 0
    for f in nc.m.functions:
        for bb in f.blocks:
            new_insts = []
            for inst in bb.instructions:
                si = inst.sync_info
                waits = list(si.on_wait) if si and si.on_wait else []
                if len(waits) > max_waits:
                    keep = waits[:max_waits]
                    extra = waits[max_waits:]
                    for i in range(0, len(extra), max_waits):
                        nop = mybir.InstNoOp(name=f"WSPLIT-{n}", ins=[], outs=[])
                        n += 1
                        nop.engine = inst.engine
                        nop.bass_nofuse = False
                        nop.debug = inst.debug
                        nop.sync_info = bass_rust.SyncInfo(
                            on_wait=extra[i : i + max_waits], on_update=[]
                        )
                        new_insts.append(nop)
                    si.on_wait = keep
                    inst.sync_info = si
                new_insts.append(inst)
            bb.instructions[:] = new_insts


_prog = None


def _build(split_waits=True):
    """split_waits=False skips the walrus wait-splitting pass (needed for
    hardware codegen, but its bare NoOps break CoreSim's event loop)."""
    global _prog
    if _prog is not None:
        return _prog

    import concourse.bass as bass
    import concourse.mybir as mybir
    import concourse.tile as tile
    from concourse.masks import make_identity

    f32 = mybir.dt.float32
    bf16 = mybir.dt.bfloat16
    EXP = mybir.ActivationFunctionType.Exp
    MULT = mybir.AluOpType.mult

    nc = bass.Bass()
    xqT_d = nc.declare_dram_parameter("xqT", [H, S], bf16, isOutput=False)
    xkvT_d = nc.declare_dram_parameter("xkvT", [H, S], bf16, isOutput=False)
    wqT_d = nc.declare_dram_parameter("wqT", [H, JL], bf16, isOutput=False)
    wkT_d = nc.declare_dram_parameter("wkT", [H, JL], bf16, isOutput=False)
    wvT_d = nc.declare_dram_parameter("wvT", [H, JL], bf16, isOutput=False)
    woT_d = nc.declare_dram_parameter("woT", [JL, H], bf16, isOutput=False)
    expbT_d = nc.declare_dram_parameter("expbT", [NH_L // 2, S, 2, S], bf16, isOutput=False)
    yT_d = nc.declare_dram_parameter("yT", [H, S], bf16, isOutput=True)

    with tile.TileContext(nc) as tc:
        with (
            tc.tile_pool(name="singles", bufs=1) as singles,
            tc.tile_pool(name="expbp", bufs=4) as expbp,
            tc.tile_pool(name="expsp", bufs=6) as expsp,
            tc.tile_pool(name="attnp", bufs=31) as attnp,
            tc.tile_pool(name="smallp", bufs=4) as smallp,
            tc.tile_pool(name="outp", bufs=4) as outp,
            tc.tile_pool(name="ps_s", bufs=2, space="PSUM") as ps_s,
            tc.tile_pool(name="ps_mm", bufs=4, space="PSUM") as ps_mm,
        ):
            xq_sb = singles.tile([P, FT, S], bf16)
            xkv_sb = singles.tile([P, FT, S], bf16)
            wq_sb = singles.tile([P, FT, JL], bf16)
            wk_sb = singles.tile([P, FT, JL], bf16)
            wv_sb = singles.tile([P, FT, JL], bf16)
            wo_sb = singles.tile([P, JC, H], bf16)
            qT_sb = singles.tile([P, JC, S], bf16)
            kT_sb = singles.tile([P, JC, S], bf16)
            v_sb = singles.tile([P, TT, NH_L, DK + 1], bf16)
            aoT_sb = singles.tile([P, JC, S], bf16)
            ident = singles.tile([P, P], bf16)

            make_identity(nc, ident)
            nc.vector.memset(v_sb[:, :, :, DK : DK + 1], 1.0)

            def load2(sb, dr, cols=None):
                drr = dr.rearrange("(n p) j -> p n j", p=P)
                for f2 in range(FT // 2):
                    s = slice(2 * f2, 2 * f2 + 2)
                    if cols is None:
                        nc.sync.dma_start(out=sb[:, s, :], in_=drr[:, s, :])
                    else:
                        nc.sync.dma_start(
                            out=sb[:, s, cols], in_=drr[:, s, cols]
                        )

            expb_tiles = {}

            def expb_fetch(h, mt):
                # one DMA fetches BOTH heads of the pair into a [P, 2, S]
                # tile: halves the sync-queue descriptor-gen ops that pace
                # the bias stream. The source AP reads two 2KB runs per
                # partition (one per head plane).
                hp = h // 2
                if (hp, mt) in expb_tiles:
                    return
                t = expbp.tile([P, 2, S], bf16, name=f"expb_{hp}_{mt}", tag="expb")
                nc.sync.dma_start(
                    out=t, in_=expbT_d[hp, mt * P : (mt + 1) * P, :, :]
                )
                expb_tiles[(hp, mt)] = t

            # DMA order matters: the PE consumes in roughly this order.
            load2(wq_sb, wqT_d)
            load2(xq_sb, xqT_d, cols=slice(0, QF))
            load2(xq_sb, xqT_d, cols=slice(QF, S))
            expb_fetch(0, 0)
            expb_fetch(1, 0)
            load2(wk_sb, wkT_d)
            load2(xkv_sb, xkvT_d, cols=slice(0, QF))
            load2(xkv_sb, xkvT_d, cols=slice(QF, S))
            expb_fetch(0, 1)
            expb_fetch(1, 1)
            expb_fetch(0, 2)
            expb_fetch(1, 2)
            expb_fetch(0, 3)
            expb_fetch(1, 3)
            load2(wv_sb, wvT_d)
            for hdt in range(JC):
                nc.sync.dma_start(
                    out=wo_sb[:, hdt, :], in_=woT_d[hdt * P : (hdt + 1) * P, :]
                )

            # HAM warm-up: tiny matmuls bridge the input-DMA window so the
            # PE clock is at 8/8 when real work starts.
            warm_ps = ps_mm.tile([P, P], f32, name="warm", tag="mm")
            for _ in range(72):
                nc.tensor.matmul(warm_ps, lhsT=ident, rhs=ident,
                                 start=True, stop=True, skip_group_check=True)

            def proj_qk_unit(jc, which, tch):
                nm, w_sb, x_sb, out_sb = (
                    ("q", wq_sb, xq_sb, qT_sb)
                    if which == 0
                    else ("k", wk_sb, xkv_sb, kT_sb)
                )
                ps = ps_mm.tile([P, QF], f32, name=f"pj{nm}_{jc}_{tch}", tag="mm")
                for ft in range(FT):
                    nc.tensor.matmul(
                        ps,
                        lhsT=w_sb[:, ft, jc * P : (jc + 1) * P],
                        rhs=x_sb[:, ft, tch * QF : (tch + 1) * QF],
                        start=(ft == 0),
                        stop=(ft == FT - 1),
                    )
                nc.vector.tensor_copy(
                    out=out_sb[:, jc, tch * QF : (tch + 1) * QF], in_=ps
                )

            def proj_v_unit(tt):
                ps = ps_mm.tile([P, QF], f32, name=f"pjv_{tt}", tag="mm")
                for ft in range(FT):
                    nc.tensor.matmul(
                        ps,
                        lhsT=xkv_sb[:, ft, tt * P : (tt + 1) * P],
                        rhs=wv_sb[:, ft, :],
                        start=(ft == 0),
                        stop=(ft == FT - 1),
                    )
                nc.vector.tensor_copy(
                    out=v_sb[:, tt, :, 0:DK],
                    in_=ps.rearrange("p (h d) -> p h d", h=NH_L),
                )

            attn_tiles = {}
            av_tiles = {}
            inc_started = set()

            def av_inc(h, qc, mts):
                # incremental attn@v accumulation (pair-3 heads): the PSUM
                # group stays open across the pair; mt order is free since
                # accumulation commutes.
                key = (h, qc)
                if key not in av_tiles:
                    av_tiles[key] = ps_mm.tile(
                        [P, QF], f32, name=f"av_{h}_{qc}", tag="mm"
                    )
                av = av_tiles[key]
                for mt in mts:
                    nc.tensor.matmul(
                        av[0 : DK + 1, :],
                        lhsT=v_sb[:, mt, h, :],
                        rhs=attn_tiles[(h, mt)][:, qc * QF : (qc + 1) * QF],
                        start=(h, qc, "started") not in inc_started,
                        stop=False,
                        skip_group_check=True,
                    )
                    inc_started.add((h, qc, "started"))

            def scores_step(hp, mt):
                g = hp * TT + mt
                for i in range(2):
                    h = 2 * hp + i
                    # per-head: both qc matmuls then the exp — head i's
                    # psum WARs only against head i's exp one step back
                    ps = ps_s.tile([P, S], f32, name=f"sc_{hp}_{mt}_{i}", tag="sc")
                    for qc in range(QC):
                        jr = i * DK
                        nc.tensor.matmul(
                            ps[:, qc * QF : (qc + 1) * QF],
                            lhsT=kT_sb[jr : jr + DK, hp, mt * P : (mt + 1) * P],
                            rhs=qT_sb[jr : jr + DK, hp, qc * QF : (qc + 1) * QF],
                            start=True,
                            stop=True,
                        )
                    at = attnp.tile([P, S], bf16, name=f"attn_{h}_{mt}", tag="attn")
                    attn_tiles[(h, mt)] = at
                    # exp lands in a scratch tile: the multiply must NOT be
                    # in-place (out==in0 drops the DVE to 1x mode, 2x cost)
                    et = expsp.tile([P, S], bf16, name=f"exps_{h}_{mt}", tag="exps")
                    nc.scalar.activation(
                        out=et, in_=ps, func=EXP, scale=0.125
                    )
                    ebt = (expb_tiles[(hp, mt)] if i == 0
                           else expb_tiles.pop((hp, mt)))
                    nc.vector.tensor_tensor(
                        out=at, in0=et, in1=ebt[:, i, :], op=MULT
                    )
                # keep the exp(bias) stream ~4 steps ahead; emitted after the
                # mults so the buffer-rotation WAR dep is already recorded
                gp = g + 4
                if gp < 4 * TT:
                    expb_fetch(2 * (gp // TT), gp % TT)
                    expb_fetch(2 * (gp // TT) + 1, gp % TT)

            rec_rows = {}

            def attn_v_A(h, qc):
                # discrete attn@v unit (heads 0-5) + denominator reciprocal.
                # The den row is DMA-reshaped to [128, 4] so the iterative
                # divide (8 cyc/elem) spreads across 128 lanes. The den copy
                # alternates DVE/ACT to balance engine load.
                av = ps_mm.tile([P, QF], f32, name=f"av_{h}_{qc}", tag="mm")
                av_tiles[(h, qc)] = av
                for mt in range(TT):
                    nc.tensor.matmul(
                        av[0 : DK + 1, :],
                        lhsT=v_sb[:, mt, h, :],
                        rhs=attn_tiles[(h, mt)][:, qc * QF : (qc + 1) * QF],
                        start=(mt == 0),
                        stop=(mt == TT - 1),
                    )
                finish_av(h, qc)

            def finish_av(h, qc):
                av = av_tiles[(h, qc)]
                den = smallp.tile([1, QF], f32, name=f"den_{h}_{qc}", tag="den")
                if h % 2 == 0:
                    nc.vector.tensor_copy(out=den, in_=av[DK : DK + 1, :])
                else:
                    nc.scalar.copy(out=den, in_=av[DK : DK + 1, :])
                den_r = smallp.tile([P, QF // P], f32, name=f"denr_{h}_{qc}", tag="denr")
                nc.gpsimd.dma_start(out=den_r, in_=den)
                rec_r = smallp.tile([P, QF // P], f32, name=f"recr_{h}_{qc}", tag="recr")
                nc.vector.reciprocal(out=rec_r, in_=den_r)
                rec = smallp.tile([1, QF], bf16, name=f"rec_{h}_{qc}", tag="rec")
                nc.gpsimd.dma_start(out=rec, in_=rec_r)
                rec_rows[(h, qc)] = rec

            def attn_v_B(hp, qc):
                # broadcast 1/den across the 64 head-dim partitions with a
                # 0-stride DMA (reads rec [1,512] 64x) — no PE involvement.
                h0, h1 = 2 * hp, 2 * hp + 1
                bcs = smallp.tile([P, QF], bf16, name=f"bcs_{hp}_{qc}", tag="bcs")
                for i, h in ((0, h0), (1, h1)):
                    nc.gpsimd.dma_start(
                        out=bcs[i * DK : (i + 1) * DK, :],
                        in_=rec_rows[(h, qc)][:, None, :].broadcast_to(
                            [1, DK, QF]
                        ),
                    )
                    nc.vector.tensor_tensor(
                        out=aoT_sb[
                            i * DK : (i + 1) * DK,
                            hp,
                            qc * QF : (qc + 1) * QF,
                        ],
                        in0=av_tiles[(h, qc)][0:DK, :],
                        in1=bcs[i * DK : (i + 1) * DK, :],
                        op=MULT,
                    )

            def wo_unit(oc, qc):
                ps = ps_mm.tile([P, QF], f32, name=f"y_{oc}_{qc}", tag="mm")
                for hdt in range(JC):
                    nc.tensor.matmul(
                        ps,
                        lhsT=wo_sb[:, hdt, oc * P : (oc + 1) * P],
                        rhs=aoT_sb[:, hdt, qc * QF : (qc + 1) * QF],
                        start=(hdt == 0),
                        stop=(hdt == JC - 1),
                    )
                ysb = outp.tile([P, QF], bf16, name=f"ysb_{oc}_{qc}", tag="y")
                # alternate engines: ACT is idle in the tail, share the load
                if oc % 2 == 0:
                    nc.vector.tensor_copy(out=ysb, in_=ps)
                else:
                    nc.scalar.copy(out=ysb, in_=ps)
                nc.sync.dma_start(
                    out=yT_d[oc * P : (oc + 1) * P, qc * QF : (qc + 1) * QF],
                    in_=ysb,
                )

            # ---- interleaved emission schedule ----
            # Pairs 0-2: scores steps paced by ACT exp (~2.3us/step) with
            # discrete filler units (projections; heads 0-5 attn@v one pair
            # behind). Pair 3: all remaining discrete units front-loaded in
            # the first steps, then heads 6/7 attn@v accumulates
            # incrementally so the post-exp tail is only the last BB chains
            # + wo units. Ordering rules: (1) av units of pair p lead pair
            # p+1's filler list (attn pool reuse); (2) av and proj PSUM
            # tiles share one 4-buffer rotation, so each A-unit's pair-B
            # (which frees its buffer via the aoT mult) must be emitted
            # before the 4th-later "mm" allocation; (3) units between
            # A(h,qc) and B(p,qc) cover the den/reciprocal chain.
            def A(h, qc):
                return lambda: attn_v_A(h, qc)

            def BB(hp, qc):
                return lambda: attn_v_B(hp, qc)

            def qk(jc, w, t):
                return lambda: proj_qk_unit(jc, w, t)

            def pv(tt):
                return lambda: proj_v_unit(tt)

            def wu(oc, qc):
                return lambda: wo_unit(oc, qc)

            # prologue PE work: q then k for head pair 0 (jc0), plus q jc1
            for tch in range(QC):
                proj_qk_unit(0, 0, tch)
            for tch in range(QC):
                proj_qk_unit(1, 0, tch)
            for tch in range(QC):
                proj_qk_unit(0, 1, tch)

            fillers_by_pair = [
                [qk(1, 1, 0), qk(1, 1, 1)]
                + [pv(tt) for tt in range(TT)]
                + [qk(2, 0, 0), qk(2, 0, 1)],
                [A(0, 0), A(1, 0), A(0, 1), qk(2, 1, 0), BB(0, 0),
                 A(1, 1), qk(2, 1, 1), BB(0, 1), qk(3, 0, 0), qk(3, 0, 1)],
                [A(2, 0), A(3, 0), A(2, 1), qk(3, 1, 0), BB(1, 0),
                 A(3, 1), qk(3, 1, 1), BB(1, 1)],
            ]
            for hp in range(3):
                fillers = fillers_by_pair[hp]
                k = 0
                for mt in range(TT):
                    scores_step(hp, mt)
                    want = (mt + 1) * len(fillers) // TT
                    while k < want:
                        fillers[k]()
                        k += 1

            # ---- pair 3: front-load discrete units, then incremental av.
            # Emission per step: scores_step, then units/av-matmuls. The
            # heads 4/5 A-units land in steps 0-1 (deps ready at pair
            # start); their BB chains follow; av(6/7) tiles then take over
            # the freed mm buffers (allocation order matches rule (2)).
            p3_units = [
                [A(4, 0), A(5, 0), A(4, 1)],             # after step 0
                [BB(2, 0), A(5, 1)],                     # after step 1
                [BB(2, 1)],                              # after step 2
            ]
            # incremental av plan: after step s (2..7), accumulate mt
            # ranges per tile; tiles allocated in order (6,0),(6,1),(7,0),
            # (7,1) after BB(2,0)/BB(2,1) free the rotation buffers.
            inc_plan = {
                3: [(6, 0, [0, 1]), (6, 1, [0, 1])],
                4: [(7, 0, [0, 1]), (7, 1, [0, 1]), (6, 0, [2, 3])],
                5: [(6, 1, [2, 3]), (7, 0, [2, 3]), (7, 1, [2, 3])],
                6: [(6, 0, [4, 5]), (6, 1, [4, 5]), (7, 0, [4, 5]),
                    (7, 1, [4, 5])],
                7: [(6, 0, [6, 7]), (6, 1, [6, 7]), (7, 0, [6, 7]),
                    (7, 1, [6, 7])],
            }
            for mt in range(TT):
                scores_step(3, mt)
                if mt < len(p3_units):
                    for u in p3_units[mt]:
                        u()
                for h, qc, mts in inc_plan.get(mt, []):
                    av_inc(h, qc, mts)
            # the incremental groups were left open (stop=False): hardware
            # doesn't need stop; the sim's group check was skipped. Finish
            # order (6,0),(7,0),(6,1),(7,1) keeps the 4-deep rec/den
            # rotation ahead of the BB readers; BB(3,1) must be emitted
            # before wu(1,0) so the wo rotation sees av(6,1)'s last reader.
            for h, qc in ((6, 0), (7, 0), (6, 1), (7, 1)):
                finish_av(h, qc)
            tail = [BB(3, 0), wu(0, 0), BB(3, 1), wu(1, 0)]
            tail += [wu(oc, 0) for oc in range(2, FT)]
            tail += [wu(oc, 1) for oc in range(FT)]
            for f in tail:
                f()

    if split_waits:
        _split_waits(nc)
    _prog = nc
    return nc


def _in_maps(x_q, x_kv, bias, Wq, Wk, Wv, Wo):
    import ml_dtypes

    bf16 = ml_dtypes.bfloat16

    def cvt(a):
        return np.ascontiguousarray(a).astype(bf16)

    bias = np.asarray(bias, np.float32)
    maps = []
    for c in range(N_CORES):
        b, g = c // 2, c % 2
        hd = slice(g * JL, (g + 1) * JL)
        hs = slice(g * NH_L, (g + 1) * NH_L)
        maps.append(
            {
                "xqT": cvt(x_q[b].T),
                "xkvT": cvt(x_kv[b].T),
                "wqT": cvt(Wq[hd, :].T),
                "wkT": cvt(Wk[hd, :].T),
                "wvT": cvt(Wv[hd, :].T),
                "woT": cvt(Wo[:, hd].T),
                "expbT": cvt(np.exp(bias[b, hs]).reshape(NH_L // 2, 2, S, S)
                             .transpose(0, 3, 1, 2)),
            }
        )
    return maps


def _postprocess(results, x_q):
    y = np.empty((B, S, H), np.float32)
    for b in range(B):
        acc = results[2 * b]["yT"].astype(np.float32) + results[2 * b + 1][
            "yT"
        ].astype(np.float32)
        y[b] = x_q[b].astype(np.float32) + acc.T
    return y


def kernel(x_q, x_kv, bias, Wq, Wk, Wv, Wo):
    x_q = np.asarray(x_q)
    nc = _build()
    maps = _in_maps(x_q, np.asarray(x_kv), np.asarray(bias), np.asarray(Wq),
                    np.asarray(Wk), np.asarray(Wv), np.asarray(Wo))
    from concourse.bass_utils import run_bass_kernel_spmd

    res = run_bass_kernel_spmd(nc, maps, list(range(N_CORES)))
    return _postprocess(res.results, x_q)


# revision 46
# speedup vs baseline: 1.0017x; 1.0017x over previous
"""Trainium2 Bass kernel: transformer block (biased attention + residual).

Reference math (B=4, S=1024, H=1024, NH=16, DK=64):
    q = x_q @ Wq.T ; k = x_kv @ Wk.T ; v = x_kv @ Wv.T   (per-head reshape)
    scores = q k^T / sqrt(DK) + bias ; attn = softmax(scores)
    out = x_q + (attn v reshaped) @ Wo.T

Sharding: 8 cores = 4 batches x 2 head-groups (8 heads each). Each core
computes its (batch, head-group) slice; the host sums the two head-group
partial outputs per batch and adds the residual.

Per-core dataflow (all matmul inputs bf16, PSUM accumulation fp32):
    qT/kT = W_g x^T            (head_dim on partitions, seq on free)
    v     = x_kv @ Wv_g.T      (seq on partitions), padded ones column
    sT[k,q] = k_h q_h^T        (per head: both qc halves, then exp —
                               the ps_s pool turns over once per step, so
                               head i's matmuls only WAR against head i's
                               exp of the previous step: a full exp-slot
                               of slack)
    eT  = exp(sT / 8)          (ACT, scale folded into the activation)
    aT  = eT * exp(bias)^T     (DVE bf16 2x; exp(bias) precomputed host)
    avT = v_aug^T aT           -> rows 0..63 = attn out^T, row 64 = denom
                               (heads 0-5: discrete 8-matmul units one
                               pair behind; heads 6-7: accumulated
                               incrementally during pair 3 so the tail
                               has no bulk attn@v left)
    aoT = avT[0:64] * bcast(1/denom)   (1/denom broadcast across the 64
                                        head-dim partitions by a 0-stride
                                        DMA; no PE involvement)
    yT  = Wo_g^T-contraction of aoT    (tail units; y copies alternate
                                        DVE/ACT)
"""

import sys

import numpy as np

for _p in ("/opt/trn_rl_repo",):
    if _p not in sys.path:
        sys.path.append(_p)

B, S, H, NH = 4, 1024, 1024, 16
DK = 64
P = 128
NH_L = 8            # heads per core
JL = NH_L * DK      # 512 local head dims per core
FT = H // P         # 8 contraction tiles for projections
TT = S // P         # 8 seq tiles
JC = JL // P        # 4 local head-dim chunks of 128
QF = 512            # matmul moving free dim (one PSUM bank of fp32)
QC = S // QF        # 2 q chunks
N_CORES = 8


def _split_waits(nc, max_waits=1):
    """This walrus build rejects instructions carrying more than ~1 sem
    wait ("Too many sync wait commands" in setupSyncWait). Hoist surplus
    waits onto same-engine NoOps spliced immediately before the carrying
    instruction — same engine position, so semantics are unchanged."""
    import bass_rust
    import concourse.mybir as mybir

    n = 0
    for f in nc.m.functions:
        for bb in f.blocks:
            new_insts = []
            for inst in bb.instructions:
                si = inst.sync_info
                waits = list(si.on_wait) if si and si.on_wait else []
                if len(waits) > max_waits:
                    keep = waits[:max_waits]
                    extra = waits[max_waits:]
                    for i in range(0, len(extra), max_waits):
                        nop = mybir.InstNoOp(name=f"WSPLIT-{n}", ins=[], outs=[])
                        n += 1
                        nop.engine = inst.engine
                        nop.bass_nofuse = False
                        nop.debug = inst.debug
                        nop.sync_info = bass_rust.SyncInfo(
                            on_wait=extra[i : i + max_waits], on_update=[]
                        )
                        new_insts.append(nop)
                    si.on_wait = keep
                    inst.sync_info = si
                new_insts.append(inst)
            bb.instructions[:] = new_insts


_prog = None


def _build(split_waits=True):
    """split_waits=False skips the walrus wait-splitting pass (needed for
    hardware codegen, but its bare NoOps break CoreSim's event loop)."""
    global _prog
    if _prog is not None:
        return _prog

    import concourse.bass as bass
    import concourse.mybir as mybir
    import concourse.tile as tile
    from concourse.masks import make_identity

    f32 = mybir.dt.float32
    bf16 = mybir.dt.bfloat16
    EXP = mybir.ActivationFunctionType.Exp
    MULT = mybir.AluOpType.mult

    nc = bass.Bass()
    xqT_d = nc.declare_dram_parameter("xqT", [H, S], bf16, isOutput=False)
    xkvT_d = nc.declare_dram_parameter("xkvT", [H, S], bf16, isOutput=False)
    wqT_d = nc.declare_dram_parameter("wqT", [H, JL], bf16, isOutput=False)
    wkT_d = nc.declare_dram_parameter("wkT", [H, JL], bf16, isOutput=False)
    wvT_d = nc.declare_dram_parameter("wvT", [H, JL], bf16, isOutput=False)
    woT_d = nc.declare_dram_parameter("woT", [JL, H], bf16, isOutput=False)
    expbT_d = nc.declare_dram_parameter("expbT", [NH_L, S, S], bf16, isOutput=False)
    yT_d = nc.declare_dram_parameter("yT", [H, S], bf16, isOutput=True)

    with tile.TileContext(nc) as tc:
        with (
            tc.tile_pool(name="singles", bufs=1) as singles,
            tc.tile_pool(name="expbp", bufs=4) as expbp,
            tc.tile_pool(name="expsp", bufs=6) as expsp,
            tc.tile_pool(name="attnp", bufs=31) as attnp,
            tc.tile_pool(name="smallp", bufs=4) as smallp,
            tc.tile_pool(name="outp", bufs=4) as outp,
            tc.tile_pool(name="ps_s", bufs=2, space="PSUM") as ps_s,
            tc.tile_pool(name="ps_mm", bufs=4, space="PSUM") as ps_mm,
        ):
            xq_sb = singles.tile([P, FT, S], bf16)
            xkv_sb = singles.tile([P, FT, S], bf16)
            wq_sb = singles.tile([P, FT, JL], bf16)
            wk_sb = singles.tile([P, FT, JL], bf16)
            wv_sb = singles.tile([P, FT, JL], bf16)
            wo_sb = singles.tile([P, JC, H], bf16)
            qT_sb = singles.tile([P, JC, S], bf16)
            kT_sb = singles.tile([P, JC, S], bf16)
            v_sb = singles.tile([P, TT, NH_L, DK + 1], bf16)
            aoT_sb = singles.tile([P, JC, S], bf16)
            ident = singles.tile([P, P], bf16)

            make_identity(nc, ident)
            nc.vector.memset(v_sb[:, :, :, DK : DK + 1], 1.0)

            def load2(sb, dr, cols=None):
                drr = dr.rearrange("(n p) j -> p n j", p=P)
                for f2 in range(FT // 2):
                    s = slice(2 * f2, 2 * f2 + 2)
                    if cols is None:
                        nc.sync.dma_start(out=sb[:, s, :], in_=drr[:, s, :])
                    else:
                        nc.sync.dma_start(
                            out=sb[:, s, cols], in_=drr[:, s, cols]
                        )

            expb_tiles = {}

            def expb_fetch(h, mt):
                # one DMA fetches BOTH heads of the pair into a [P, 2, S]
                # tile: halves the sync-queue descriptor-gen ops that pace
                # the bias stream. The source AP reads two 2KB runs per
                # partition (one per head plane).
                hp = h // 2
                if (hp, mt) in expb_tiles:
                    return
                t = expbp.tile([P, 2, S], bf16, name=f"expb_{hp}_{mt}", tag="expb")
                nc.sync.dma_start(
                    out=t,
                    in_=expbT_d[2 * hp : 2 * hp + 2, mt * P : (mt + 1) * P, :]
                    .rearrange("h p j -> p h j"),
                )
                expb_tiles[(hp, mt)] = t

            # DMA order matters: the PE consumes in roughly this order.
            load2(wq_sb, wqT_d)
            load2(xq_sb, xqT_d, cols=slice(0, QF))
            load2(xq_sb, xqT_d, cols=slice(QF, S))
            expb_fetch(0, 0)
            expb_fetch(1, 0)
            load2(wk_sb, wkT_d)
            load2(xkv_sb, xkvT_d, cols=slice(0, QF))
            load2(xkv_sb, xkvT_d, cols=slice(QF, S))
            expb_fetch(0, 1)
            expb_fetch(1, 1)
            expb_fetch(0, 2)
            expb_fetch(1, 2)
            expb_fetch(0, 3)
            expb_fetch(1, 3)
            load2(wv_sb, wvT_d)
            for hdt in range(JC):
                nc.sync.dma_start(
                    out=wo_sb[:, hdt, :], in_=woT_d[hdt * P : (hdt + 1) * P, :]
                )

            # HAM warm-up: tiny matmuls bridge the input-DMA window so the
            # PE clock is at 8/8 when real work starts.
            warm_ps = ps_mm.tile([P, P], f32, name="warm", tag="mm")
            for _ in range(72):
                nc.tensor.matmul(warm_ps, lhsT=ident, rhs=ident,
                                 start=True, stop=True, skip_group_check=True)

            def proj_qk_unit(jc, which, tch):
                nm, w_sb, x_sb, out_sb = (
                    ("q", wq_sb, xq_sb, qT_sb)
                    if which == 0
                    else ("k", wk_sb, xkv_sb, kT_sb)
                )
                ps = ps_mm.tile([P, QF], f32, name=f"pj{nm}_{jc}_{tch}", tag="mm")
                for ft in range(FT):
                    nc.tensor.matmul(
                        ps,
                        lhsT=w_sb[:, ft, jc * P : (jc + 1) * P],
                        rhs=x_sb[:, ft, tch * QF : (tch + 1) * QF],
                        start=(ft == 0),
                        stop=(ft == FT - 1),
                    )
                nc.vector.tensor_copy(
                    out=out_sb[:, jc, tch * QF : (tch + 1) * QF], in_=ps
                )

            def proj_v_unit(tt):
                ps = ps_mm.tile([P, QF], f32, name=f"pjv_{tt}", tag="mm")
                for ft in range(FT):
                    nc.tensor.matmul(
                        ps,
                        lhsT=xkv_sb[:, ft, tt * P : (tt + 1) * P],
                        rhs=wv_sb[:, ft, :],
                        start=(ft == 0),
                        stop=(ft == FT - 1),
                    )
                nc.vector.tensor_copy(
                    out=v_sb[:, tt, :, 0:DK],
                    in_=ps.rearrange("p (h d) -> p h d", h=NH_L),
                )

            attn_tiles = {}
            av_tiles = {}
            inc_started = set()

            def av_inc(h, qc, mts):
                # incremental attn@v accumulation (pair-3 heads): the PSUM
                # group stays open across the pair; mt order is free since
                # accumulation commutes.
                key = (h, qc)
                if key not in av_tiles:
                    av_tiles[key] = ps_mm.tile(
                        [P, QF], f32, name=f"av_{h}_{qc}", tag="mm"
                    )
                av = av_tiles[key]
                for mt in mts:
                    nc.tensor.matmul(
                        av[0 : DK + 1, :],
                        lhsT=v_sb[:, mt, h, :],
                        rhs=attn_tiles[(h, mt)][:, qc * QF : (qc + 1) * QF],
                        start=(h, qc, "started") not in inc_started,
                        stop=False,
                        skip_group_check=True,
                    )
                    inc_started.add((h, qc, "started"))

            def scores_step(hp, mt):
                g = hp * TT + mt
                for i in range(2):
                    h = 2 * hp + i
                    # per-head: both qc matmuls then the exp — head i's
                    # psum WARs only against head i's exp one step back
                    ps = ps_s.tile([P, S], f32, name=f"sc_{hp}_{mt}_{i}", tag="sc")
                    for qc in range(QC):
                        jr = i * DK
                        nc.tensor.matmul(
                            ps[:, qc * QF : (qc + 1) * QF],
                            lhsT=kT_sb[jr : jr + DK, hp, mt * P : (mt + 1) * P],
                            rhs=qT_sb[jr : jr + DK, hp, qc * QF : (qc + 1) * QF],
                            start=True,
                            stop=True,
                        )
                    at = attnp.tile([P, S], bf16, name=f"attn_{h}_{mt}", tag="attn")
                    attn_tiles[(h, mt)] = at
                    # exp lands in a scratch tile: the multiply must NOT be
                    # in-place (out==in0 drops the DVE to 1x mode, 2x cost)
                    et = expsp.tile([P, S], bf16, name=f"exps_{h}_{mt}", tag="exps")
                    nc.scalar.activation(
                        out=et, in_=ps, func=EXP, scale=0.125
                    )
                    ebt = (expb_tiles[(hp, mt)] if i == 0
                           else expb_tiles.pop((hp, mt)))
                    nc.vector.tensor_tensor(
                        out=at, in0=et, in1=ebt[:, i, :], op=MULT
                    )
                # keep the exp(bias) stream ~4 steps ahead; emitted after the
                # mults so the buffer-rotation WAR dep is already recorded
                gp = g + 4
                if gp < 4 * TT:
                    expb_fetch(2 * (gp // TT), gp % TT)
                    expb_fetch(2 * (gp // TT) + 1, gp % TT)

            rec_rows = {}

            def attn_v_A(h, qc):
                # discrete attn@v unit (heads 0-5) + denominator reciprocal.
                # The den row is DMA-reshaped to [128, 4] so the iterative
                # divide (8 cyc/elem) spreads across 128 lanes. The den copy
                # alternates DVE/ACT to balance engine load.
                av = ps_mm.tile([P, QF], f32, name=f"av_{h}_{qc}", tag="mm")
                av_tiles[(h, qc)] = av
                for mt in range(TT):
                    nc.tensor.matmul(
                        av[0 : DK + 1, :],
                        lhsT=v_sb[:, mt, h, :],
                        rhs=attn_tiles[(h, mt)][:, qc * QF : (qc + 1) * QF],
                        start=(mt == 0),
                        stop=(mt == TT - 1),
                    )
                finish_av(h, qc)

            def finish_av(h, qc):
                av = av_tiles[(h, qc)]
                den = smallp.tile([1, QF], f32, name=f"den_{h}_{qc}", tag="den")
                if h % 2 == 0:
                    nc.vector.tensor_copy(out=den, in_=av[DK : DK + 1, :])
                else:
                    nc.scalar.copy(out=den, in_=av[DK : DK + 1, :])
                den_r = smallp.tile([P, QF // P], f32, name=f"denr_{h}_{qc}", tag="denr")
                nc.gpsimd.dma_start(out=den_r, in_=den)
                rec_r = smallp.tile([P, QF // P], f32, name=f"recr_{h}_{qc}", tag="recr")
                nc.vector.reciprocal(out=rec_r, in_=den_r)
                rec = smallp.tile([1, QF], bf16, name=f"rec_{h}_{qc}", tag="rec")
                nc.gpsimd.dma_start(out=rec, in_=rec_r)
                rec_rows[(h, qc)] = rec

            def attn_v_B(hp, qc):
                # broadcast 1/den across the 64 head-dim partitions with a
                # 0-stride DMA (reads rec [1,512] 64x) — no PE involvement.
                h0, h1 = 2 * hp, 2 * hp + 1
                bcs = smallp.tile([P, QF], bf16, name=f"bcs_{hp}_{qc}", tag="bcs")
                for i, h in ((0, h0), (1, h1)):
                    nc.gpsimd.dma_start(
                        out=bcs[i * DK : (i + 1) * DK, :],
                        in_=rec_rows[(h, qc)][:, None, :].broadcast_to(
                            [1, DK, QF]
                        ),
                    )
                    nc.vector.tensor_tensor(
                        out=aoT_sb[
                            i * DK : (i + 1) * DK,
                            hp,
                            qc * QF : (qc + 1) * QF,
                        ],
                        in0=av_tiles[(h, qc)][0:DK, :],
                        in1=bcs[i * DK : (i + 1) * DK, :],
                        op=MULT,
                    )

            def wo_unit(oc, qc):
                ps = ps_mm.tile([P, QF], f32, name=f"y_{oc}_{qc}", tag="mm")
                for hdt in range(JC):
                    nc.tensor.matmul(
                        ps,
                        lhsT=wo_sb[:, hdt, oc * P : (oc + 1) * P],
                        rhs=aoT_sb[:, hdt, qc * QF : (qc + 1) * QF],
                        start=(hdt == 0),
                        stop=(hdt == JC - 1),
                    )
                ysb = outp.tile([P, QF], bf16, name=f"ysb_{oc}_{qc}", tag="y")
                # alternate engines: ACT is idle in the tail, share the load
                if oc % 2 == 0:
                    nc.vector.tensor_copy(out=ysb, in_=ps)
                else:
                    nc.scalar.copy(out=ysb, in_=ps)
                nc.sync.dma_start(
                    out=yT_d[oc * P : (oc + 1) * P, qc * QF : (qc + 1) * QF],
                    in_=ysb,
                )

            # ---- interleaved emission schedule ----
            # Pairs 0-2: scores steps paced by ACT exp (~2.3us/step) with
            # discrete filler units (projections; heads 0-5 attn@v one pair
            # behind). Pair 3: all remaining discrete units front-loaded in
            # the first steps, then heads 6/7 attn@v accumulates
            # incrementally so the post-exp tail is only the last BB chains
            # + wo units. Ordering rules: (1) av units of pair p lead pair
            # p+1's filler list (attn pool reuse); (2) av and proj PSUM
            # tiles share one 4-buffer rotation, so each A-unit's pair-B
            # (which frees its buffer via the aoT mult) must be emitted
            # before the 4th-later "mm" allocation; (3) units between
            # A(h,qc) and B(p,qc) cover the den/reciprocal chain.
            def A(h, qc):
                return lambda: attn_v_A(h, qc)

            def BB(hp, qc):
                return lambda: attn_v_B(hp, qc)

            def qk(jc, w, t):
                return lambda: proj_qk_unit(jc, w, t)

            def pv(tt):
                return lambda: proj_v_unit(tt)

            def wu(oc, qc):
                return lambda: wo_unit(oc, qc)

            # prologue PE work: q then k for head pair 0 (jc0), plus q jc1
            for tch in range(QC):
                proj_qk_unit(0, 0, tch)
            for tch in range(QC):
                proj_qk_unit(1, 0, tch)
            for tch in range(QC):
                proj_qk_unit(0, 1, tch)

            fillers_by_pair = [
                [qk(1, 1, 0), qk(1, 1, 1)]
                + [pv(tt) for tt in range(TT)]
                + [qk(2, 0, 0), qk(2, 0, 1)],
                [A(0, 0), A(1, 0), A(0, 1), qk(2, 1, 0), BB(0, 0),
                 A(1, 1), qk(2, 1, 1), BB(0, 1), qk(3, 0, 0), qk(3, 0, 1)],
                [A(2, 0), A(3, 0), A(2, 1), qk(3, 1, 0), BB(1, 0),
                 A(3, 1), qk(3, 1, 1), BB(1, 1)],
            ]
            for hp in range(3):
                fillers = fillers_by_pair[hp]
                k = 0
                for mt in range(TT):
                    scores_step(hp, mt)
                    want = (mt + 1) * len(fillers) // TT
                    while k < want:
                        fillers[k]()
                        k += 1

            # ---- pair 3: front-load discrete units, then incremental av.
            # Emission per step: scores_step, then units/av-matmuls. The
            # heads 4/5 A-units land in steps 0-1 (deps ready at pair
            # start); their BB chains follow; av(6/7) tiles then take over
            # the freed mm buffers (allocation order matches rule (2)).
            p3_units = [
                [A(4, 0), A(5, 0), A(4, 1)],             # after step 0
                [BB(2, 0), A(5, 1)],                     # after step 1
                [BB(2, 1)],                              # after step 2
            ]
            # incremental av plan: after step s (2..7), accumulate mt
            # ranges per tile; tiles allocated in order (6,0),(6,1),(7,0),
            # (7,1) after BB(2,0)/BB(2,1) free the rotation buffers.
            inc_plan = {
                3: [(6, 0, [0, 1]), (6, 1, [0, 1])],
                4: [(7, 0, [0, 1]), (7, 1, [0, 1]), (6, 0, [2, 3])],
                5: [(6, 1, [2, 3]), (7, 0, [2, 3]), (7, 1, [2, 3])],
                6: [(6, 0, [4, 5]), (6, 1, [4, 5]), (7, 0, [4, 5]),
                    (7, 1, [4, 5])],
                7: [(6, 0, [6, 7]), (6, 1, [6, 7]), (7, 0, [6, 7]),
                    (7, 1, [6, 7])],
            }
            for mt in range(TT):
                scores_step(3, mt)
                if mt < len(p3_units):
                    for u in p3_units[mt]:
                        u()
                for h, qc, mts in inc_plan.get(mt, []):
                    av_inc(h, qc, mts)
            # the incremental groups were left open (stop=False): hardware
            # doesn't need stop; the sim's group check was skipped. Finish
            # order (6,0),(7,0),(6,1),(7,1) keeps the 4-deep rec/den
            # rotation ahead of the BB readers; BB(3,1) must be emitted
            # before wu(1,0) so the wo rotation sees av(6,1)'s last reader.
            for h, qc in ((6, 0), (7, 0), (6, 1), (7, 1)):
                finish_av(h, qc)
            tail = [BB(3, 0), wu(0, 0), BB(3, 1), wu(1, 0)]
            tail += [wu(oc, 0) for oc in range(2, FT)]
            tail += [wu(oc, 1) for oc in range(FT)]
            for f in tail:
                f()

    if split_waits:
        _split_waits(nc)
    _prog = nc
    return nc


def _in_maps(x_q, x_kv, bias, Wq, Wk, Wv, Wo):
    import ml_dtypes

    bf16 = ml_dtypes.bfloat16

    def cvt(a):
        return np.ascontiguousarray(a).astype(bf16)

    bias = np.asarray(bias, np.float32)
    maps = []
    for c in range(N_CORES):
        b, g = c // 2, c % 2
        hd = slice(g * JL, (g + 1) * JL)
        hs = slice(g * NH_L, (g + 1) * NH_L)
        maps.append(
            {
                "xqT": cvt(x_q[b].T),
                "xkvT": cvt(x_kv[b].T),
                "wqT": cvt(Wq[hd, :].T),
                "wkT": cvt(Wk[hd, :].T),
                "wvT": cvt(Wv[hd, :].T),
                "woT": cvt(Wo[:, hd].T),
                "expbT": cvt(np.exp(bias[b, hs]).swapaxes(1, 2)),
            }
        )
    return maps


def _postprocess(results, x_q):
    y = np.empty((B, S, H), np.float32)
    for b in range(B):
        acc = results[2 * b]["yT"].astype(np.float32) + results[2 * b + 1][
            "yT"
        ].astype(np.float32)
        y[b] = x_q[b].astype(np.float32) + acc.T
    return y


def kernel(x_q, x_kv, bias, Wq, Wk, Wv, Wo):
    x_q = np.asarray(x_q)
    nc = _build()
    maps = _in_maps(x_q, np.asarray(x_kv), np.asarray(bias), np.asarray(Wq),
                    np.asarray(Wk), np.asarray(Wv), np.asarray(Wo))
    from concourse.bass_utils import run_bass_kernel_spmd

    res = run_bass_kernel_spmd(nc, maps, list(range(N_CORES)))
    return _postprocess(res.results, x_q)
